# revision 15
# baseline (speedup 1.0000x reference)
"""Chamfer-distance (CDLoss) Trainium2 Bass kernel — exp-remap softmin hybrid.

Problem: srcs, tgts [B=8, D=3, N=4096] fp32.
  P[b,i,j] = |s_i|^2 + |t_j|^2 - 2 s_i.t_j
  out = min(P, axis=1).mean() + min(P, axis=2).mean()   (scalar fp32)

Strategy (data-parallel over B across 8 NeuronCores, one batch per core):
  The 4096x4096 distance matrix is produced tile-by-tile on TensorE
  (bf16 hi/lo-split features, K=18, [128,2048]-column units, PSUM f32)
  exactly like the classical baseline.  The classical bottleneck was
  VectorE (row+col min-trees, ~147us busy); this version moves the row
  reduction into the Activation engine's PSUM drain:

  * Act applies E = exp((beta - P)/T) (bias=beta/T, scale=-1/T) while
    draining PSUM -> SBUF bf16.  Cost is identical to the plain cast
    (measured 1967ns/tile; the exp table is warmed at t=0), and
    `accum_out` yields per-partition row sums Sum_j E for free
    (+182ns/tile): row softmin = beta - T*ln(sum), computed on host.
    The row min-tree is gone.
  * The col direction stays a classical pair tree, but in E-space: E
    is a monotone remap of P, so col max of E == col min of P (exact
    to bf16 rounding).  VectorE maxes the 7 exp'd tiles per block into
    a [128,2048] slab that is shipped to DRAM as-is; the host does the
    final max over the 8 slabs x 128 partitions (no device transpose).
  * 12 "direct" units (j==0 every block, plus j==4 in odd blocks;
    m in {0,8,12,16,24,28} x both halves) skip Act: VectorE drains
    their PSUM directly with a 32:1+64:1 tensor_reduce min (rows,
    exact f16) and a mixed f32/f16 TT min into a raw per-half col
    accumulator, rebalancing ~25us of Act work onto VectorE slack.
    The previous block's col-tail group is emitted after the j==0
    drain, so drains never queue behind DVE ops that wait on Act
    (engine queues are in-order).  The LAST block accumulates its col
    max incrementally so only one 1.2us op trails the final exp.

  Numerics: T=1e-3.  exp underflow flushes far pairs (harmless for
  min); rows/cols whose true min exceeds beta+~85T can flush/overflow
  - they self-detect on host (nonfinite / threshold) and are
  recomputed exactly there (~160 rows + ~160 cols per batch, ~60
  Mflops).  Measured end-to-end rel err 2.9e-3 vs the 2e-2 gate.

  Per-core outputs:
    rs_acc [128, 64]    f32  row exp-sums per (h,m) unit (exp'd units)
    rdir   [128, 12]    f16  exact row mins of the direct units
    colE   [128, 16384] bf16 per-(half,block) col-max-of-E slabs
    colR   [128, 4096]  f16  per-half raw col mins over direct units

  HW exec ~151us warm (from the 209us/176us classical baseline); Act
  is the pacing engine (~95% busy), VectorE ~74%.  Beware ~20%
  run-to-run HW variance (p-state).
"""

import numpy as np
import ml_dtypes

_BF16 = ml_dtypes.bfloat16

# Problem geometry (hardcoded per contest contract).
_B = 8
_D = 3
_N = 4096
_P = 128              # partitions / sources per m-tile
_K = 18               # feature rows (see _features)
_NCORES = 8
_CHUNK = 2048         # PSUM chunk columns (4 banks) == column half

_D12 = True           # 12 direct units (j0 all blocks + j4 odd blocks)
_T = 1.0e-3           # softmin temperature
_SCALE = -1.0 / _T
_ROW_THR = 85.0 * _T  # host fallback threshold above beta
_COL_THR = 80.0 * _T

_prog_cache = {}

# test-harness knobs (the grading harness just calls kernel() and never
# touches these; default is the fast no-trace path)
TRACE = False
TRACE_CORES = [0]
LAST_RESULTS = None


def _build_program(n_pts=_N):
    import concourse.mybir as mybir
    import concourse.tile as tile
    from concourse import bacc

    P = _P
    MT = n_pts // P              # 32 m-tiles of 128 sources
    K = _K
    NH = n_pts // _CHUNK         # 2 column halves
    NBLK = MT // 8               # 4 eight-unit blocks per half
    f32 = mybir.dt.float32
    f16 = mybir.dt.float16
    bf16 = mybir.dt.bfloat16
    MIN = mybir.AluOpType.min
    MAX = mybir.AluOpType.max
    AX = mybir.AxisListType.X
    EXP = mybir.ActivationFunctionType.Exp

    nc = bacc.Bacc("TRN2", target_bir_lowering=False, debug=False,
                   num_devices=_NCORES)

    dram_w = nc.dram_tensor("w", [K, n_pts], bf16, kind="ExternalInput")
    dram_r = nc.dram_tensor("r", [K, n_pts], bf16, kind="ExternalInput")
    dram_bv = nc.dram_tensor("bv", [P, 1], f32, kind="ExternalInput")
    dram_rs = nc.dram_tensor("rs_acc", [P, NH * MT], f32,
                             kind="ExternalOutput")
    nrd = 12 if _D12 else 8
    dram_rd = nc.dram_tensor("rdir", [P, nrd], f16,
                             kind="ExternalOutput")
    dram_ce = nc.dram_tensor("colE", [P, NH * NBLK * _CHUNK], bf16,
                             kind="ExternalOutput")
    dram_cr = nc.dram_tensor("colR", [P, n_pts], f16,
                             kind="ExternalOutput")

    with tile.TileContext(nc) as tc:
        with (
            tc.tile_pool(name="const", bufs=1) as cpool,
            tc.tile_pool(name="ring", bufs=2) as ringpool,
            tc.tile_pool(name="stage", bufs=1) as spool,
            tc.tile_pool(name="outs", bufs=1) as rpool,
            tc.tile_pool(name="psum", bufs=2, space="PSUM") as ppool,
        ):
            # Prologue: feature loads replicated into 4 PE row groups so
            # consecutive matmuls rotate groups (LDWEIGHTS overlaps
            # MATMUL).  Priority slices cover unit 0's operands so its
            # matmuls start early; bulk loads follow.  The Act queue is
            # kept DMA-free (Act is the bottleneck engine).
            sbW = cpool.tile([128, n_pts], bf16, tag="sbW", name="sbW")
            sbR = cpool.tile([128, n_pts], bf16, tag="sbR", name="sbR")
            bv = cpool.tile([P, 1], f32, tag="bv", name="bv")
            warm = cpool.tile([P, 1], f32, tag="warm", name="warm")
            # Matmul q of EVERY half-0 unit reads the same diagonal
            # slice R[32q:32q+K, 512q:512q+512], and units 0-1 read
            # W[32q:32q+K, 0:256]: priority-load exactly those (8 small
            # DMAs split over two queues) so the pipeline starts ~5us
            # earlier; bulk loads cover the rest and exclude the
            # priority ranges to keep the dependencies small.
            for q in range(4):
                qq = nc.sync if q < 2 else nc.scalar
                qq.dma_start(sbW[32 * q:32 * q + K, 0:2 * P],
                             dram_w[:, 0:2 * P])
                qq.dma_start(sbR[32 * q:32 * q + K, 512 * q:512 * (q + 1)],
                             dram_r[:, 512 * q:512 * (q + 1)])
            nc.sync.dma_start(bv[:], dram_bv[:])
            # warm the Act exp table at t=0 so the first real exp
            # doesn't eat the 1.3us ACT_TABLE_LOAD
            nc.vector.memset(warm[:], 0.0)
            nc.scalar.activation(warm[:], warm[:],
                                 mybir.ActivationFunctionType.Exp,
                                 bias=0.0, scale=1.0)
            for g in range(4):
                nc.sync.dma_start(sbW[32 * g:32 * g + K, 2 * P:],
                                  dram_w[:, 2 * P:])
                if g > 0:
                    nc.sync.dma_start(sbR[32 * g:32 * g + K, 0:512 * g],
                                      dram_r[:, 0:512 * g])
                nc.sync.dma_start(
                    sbR[32 * g:32 * g + K, 512 * (g + 1):],
                    dram_r[:, 512 * (g + 1):])

            V = nc.vector
            S = nc.scalar

            RS = rpool.tile([P, NH * MT], f32, tag="RS", name="RS")
            RD = rpool.tile([P, nrd], f16, tag="RD", name="RD")
            CR = rpool.tile([P, n_pts], f16, tag="CR", name="CR")

            pending_tail = [None]

            def emit_tail():
                if pending_tail[0] is not None:
                    pending_tail[0]()
                    pending_tail[0] = None

            # direct units per block: j==0 always; odd blocks also j==4
            # (d=12 total -> Act and Vector both ~balanced).  The
            # per-block col-tail group (C2b + max tree + slab DMA) is
            # deferred until just after the NEXT block's first drain so
            # the drain never queues behind ops that wait on Act.
            dir_ctr = [0]
            for h in range(NH):
                cr_h = CR[:, h * _CHUNK:(h + 1) * _CHUNK]
                for blk in range(NBLK):
                    dirs = (0, 4) if (_D12 and blk % 2 == 1) else (0,)
                    expjs = [j for j in range(8) if j not in dirs]
                    ring0 = ringpool.tile([P, 4, _CHUNK], bf16,
                                          tag="ring0", name="ring0")
                    nring1 = len(expjs) - 4
                    ring1 = ringpool.tile([P, nring1, _CHUNK], bf16,
                                          tag=f"ring1_{nring1}",
                                          name="ring1")
                    C2a = ringpool.tile([P, 2, _CHUNK], bf16,
                                        tag="C2a", name="C2a")
                    CCs = ringpool.tile([P, _CHUNK], bf16, tag="CCs",
                                        name="CCs")
                    last = h == NH - 1 and blk == NBLK - 1
                    # last block: incremental col max so only one max op
                    # (not the whole tail group) follows the final exp
                    Xl = None
                    if last:
                        Xl = spool.tile([P, _CHUNK], bf16, tag="Xl",
                                        name="Xl")
                    nexp = [0]
                    firstdir = blk == 0
                    for j in range(8):
                        m = blk * 8 + j
                        u = h * MT + m
                        ps = ppool.tile([P, 64, 32], f32, tag="ps")
                        for q in range(4):
                            # row group rotates per 512-col matmul so
                            # LDWEIGHTS overlaps in-flight MATMULs.  In
                            # the very first block each unit pins one
                            # group so unit j depends only on the j-th
                            # prologue DMA pair.
                            g = q
                            col = _CHUNK * h + 512 * q
                            nc.tensor.matmul(
                                ps[:, 16 * q:16 * (q + 1), :],
                                sbW[32 * g:32 * g + K, m * P:(m + 1) * P],
                                sbR[32 * g:32 * g + K, col:col + 512],
                                start=True, stop=True,
                                tile_position=(32 * g, 0),
                            )
                        if j in dirs:
                            # direct unit: VectorE drains PSUM.  Rows:
                            # 32:1 reduce then 64:1 (exact min, f16).
                            # Cols: mixed f32/f16 TT min into the raw
                            # per-half accumulator (copy to init).
                            diridx = dir_ctr[0]
                            dir_ctr[0] += 1
                            rd1 = spool.tile([P, 64], f16, tag="rd1")
                            V.tensor_reduce(rd1[:], ps[:], axis=AX, op=MIN)
                            if firstdir:
                                V.tensor_copy(cr_h, ps[:, :, :])
                                firstdir = False
                            else:
                                V.tensor_tensor(cr_h, ps[:, :, :], cr_h,
                                                op=MIN)
                            V.tensor_reduce(RD[:, diridx:diridx + 1],
                                            rd1[:], axis=AX, op=MIN)
                            if j == 0:
                                emit_tail()
                        else:
                            k = expjs.index(j)
                            ring, jj = (ring0, k) if k < 4 else (ring1,
                                                                 k - 4)
                            S.activation(ring[:, jj, :], ps[:, :, :], EXP,
                                         bias=bv[:], scale=_SCALE,
                                         accum_out=RS[:, u:u + 1])
                            if last:
                                nexp[0] += 1
                                if nexp[0] == 2:
                                    V.tensor_tensor(Xl[:], ring0[:, 0, :],
                                                    ring0[:, 1, :],
                                                    op=MAX)
                                elif nexp[0] > 2:
                                    src_t = (ring0[:, k, :] if k < 4
                                             else ring1[:, k - 4, :])
                                    V.tensor_tensor(Xl[:], Xl[:], src_t,
                                                    op=MAX)
                        if not last and j == expjs[3]:
                            V.tensor_tensor(C2a[:], ring0[:, 0:4:2, :],
                                            ring0[:, 1:4:2, :], op=MAX)
                    if last:
                        slab = (h * NBLK + blk) * _CHUNK
                        nc.sync.dma_start(dram_ce[:, slab:slab + _CHUNK],
                                          Xl[:])
                        continue
                    # tail group: C2b + max tree + slab DMA, deferred
                    def make_tail(h=h, blk=blk, nring1=nring1,
                                  ring1=ring1, C2a=C2a, CCs=CCs):
                        def tail():
                            C2b = spool.tile([P, _CHUNK], bf16,
                                             tag="C2b")
                            C4a = spool.tile([P, _CHUNK], bf16,
                                             tag="C4a")
                            V.tensor_tensor(C2b[:], ring1[:, 0, :],
                                            ring1[:, 1, :], op=MAX)
                            V.tensor_tensor(C4a[:], C2a[:, 0, :],
                                            C2a[:, 1, :], op=MAX)
                            if nring1 == 3:
                                C4b = spool.tile([P, _CHUNK], bf16,
                                                 tag="C4b")
                                V.tensor_tensor(C4b[:], C2b[:],
                                                ring1[:, 2, :], op=MAX)
                                V.tensor_tensor(CCs[:], C4a[:], C4b[:],
                                                op=MAX)
                            else:
                                V.tensor_tensor(CCs[:], C4a[:], C2b[:],
                                                op=MAX)
                            slab = (h * NBLK + blk) * _CHUNK
                            nc.sync.dma_start(
                                dram_ce[:, slab:slab + _CHUNK], CCs[:])
                        return tail
                    pending_tail[0] = make_tail()
                nc.sync.dma_start(dram_cr[:, h * _CHUNK:(h + 1) * _CHUNK],
                                  cr_h)
            emit_tail()
            nc.sync.dma_start(dram_rs[:], RS[:])
            nc.sync.dma_start(dram_rd[:], RD[:])

    nc.compile()
    return nc


def _get_program(n_pts=_N):
    if n_pts not in _prog_cache:
        _prog_cache[n_pts] = _build_program(n_pts)
    return _prog_cache[n_pts]


def _split_bf16(x32):
    """x32 fp32 -> (hi, lo) bf16 with hi+lo ~= x to ~2^-18 rel."""
    hi = x32.astype(_BF16)
    lo = (x32 - hi.astype(np.float32)).astype(_BF16)
    return hi, lo


def _split3(x64):
    """fp64 vector -> 3 bf16 terms summing to x to ~2^-27 rel."""
    t0 = x64.astype(_BF16)
    r = x64 - t0.astype(np.float64)
    t1 = r.astype(_BF16)
    r2 = r - t1.astype(np.float64)
    t2 = r2.astype(_BF16)
    return t0, t1, t2


def _features(q, c, n_pts):
    """Feature tensors for the distance matmul.

    q: query points  [3, N] fp32; c: candidate points [3, N] fp32.
    Returns (W [18, N] bf16, R [18, N] bf16) with
      (W.T @ R)[i, j] ~= |q~_i - c~_j|^2
    with ~ the bf16-split (hi+lo) values, exact to ~2e-6.
    """
    q_hi, q_lo = _split_bf16(q)
    c_hi, c_lo = _split_bf16(c)
    q_t = q_hi.astype(np.float32) + q_lo.astype(np.float32)
    c_t = c_hi.astype(np.float32) + c_lo.astype(np.float32)

    U = (c_t.astype(np.float64) ** 2).sum(axis=0)   # candidate norms
    u0, u1, u2 = _split3(U)
    V = (q_t.astype(np.float64) ** 2).sum(axis=0)   # query norms
    v0, v1, v2 = _split3(V)

    m2q_hi = (-2.0 * q_hi.astype(np.float32)).astype(_BF16)
    m2q_lo = (-2.0 * q_lo.astype(np.float32)).astype(_BF16)
    ones = np.ones(n_pts, dtype=_BF16)

    Wg = np.concatenate([
        m2q_hi, m2q_hi, m2q_lo, m2q_lo,
        np.stack([ones, ones, ones]),
        np.stack([v0, v1, v2]),
    ], axis=0).astype(_BF16)              # [18, N]
    Rg = np.concatenate([
        c_hi, c_lo, c_hi, c_lo,
        np.stack([u0, u1, u2]),
        np.stack([ones, ones, ones]),
    ], axis=0).astype(_BF16)              # [18, N]

    return Wg, Rg


def _host_prep(srcs_b, tgts_b):
    """Features + global softmin bias beta for one batch."""
    W, R = _features(srcs_b, tgts_b, _N)
    s = srcs_b.astype(np.float64)          # [3, N]
    t = tgts_b.astype(np.float64)
    rs = (s * s).sum(0)                    # |s_i|^2
    rt = (t * t).sum(0)
    # subsample estimate of per-row mins (stride 4 -> 1024 targets)
    tsub = t[:, ::4]
    cross = s.T @ tsub                     # [N, N/4]
    psub = rs[:, None] + rt[None, ::4] - 2.0 * cross
    bi = psub.min(axis=1)
    beta = float(bi.min())
    bvec = np.full((_P, 1), beta / _T, dtype=np.float32)
    return W, R, bvec, beta, rs, rt


def _host_post(res_b, beta, rs, rt, srcs_b, tgts_b):
    """Recover row/col minima for one batch; exact host fallback for
    flushed/overflowed entries."""
    s = srcs_b.astype(np.float64)
    t = tgts_b.astype(np.float64)
    MT = _N // _P

    rs_acc = res_b["rs_acc"].astype(np.float64)   # [128, 64]
    rdir = res_b["rdir"].astype(np.float64)
    colE = res_b["colE"].astype(np.float64)       # [128, 8*2048]
    colR = res_b["colR"].astype(np.float64)       # [128, 4096]

    # --- rows ---
    M_DIR = [0, 8, 12, 16, 24, 28] if _D12 else [0, 8, 16, 24]
    rowvals = np.empty(_N)
    bad_rows = []
    for m in range(MT):
        sl = slice(m * _P, (m + 1) * _P)
        if m in M_DIR:
            d = M_DIR.index(m)
            nd = len(M_DIR)
            rowvals[sl] = np.minimum(rdir[:, d], rdir[:, nd + d])
        else:
            Rsum = rs_acc[:, m] + rs_acc[:, MT + m]
            with np.errstate(divide="ignore", invalid="ignore"):
                v = beta - _T * np.log(Rsum)
            bad = ~np.isfinite(v) | (v > beta + _ROW_THR)
            v = np.where(bad, np.inf, v)
            rowvals[sl] = v
            if bad.any():
                bad_rows.extend(m * _P + np.nonzero(bad)[0])
    if bad_rows:
        idx = np.asarray(bad_rows)
        d2 = (rs[idx, None] + rt[None, :]
              - 2.0 * (s[:, idx].T @ t))
        rowvals[idx] = d2.min(axis=1)

    # --- cols ---
    # colE slabs: (h*4+blk)*2048 + c -> target column 2048h + c
    slabs = colE.reshape(128, 2, 4, _CHUNK)       # [p, h, blk, c]
    maxE = slabs.max(axis=(0, 2)).reshape(-1)     # [4096]
    with np.errstate(divide="ignore", invalid="ignore"):
        vE = beta - _T * np.log(maxE)
    vR = colR.min(axis=0)
    colvals = np.minimum(np.where(np.isfinite(vE), vE, np.inf), vR)
    bad_c = ~np.isfinite(colvals) | (colvals > beta + _COL_THR)
    if bad_c.any():
        jdx = np.nonzero(bad_c)[0]
        d2 = (rs[:, None] + rt[jdx][None, :]
              - 2.0 * (s.T @ t[:, jdx]))
        colvals[jdx] = d2.min(axis=0)

    return rowvals.mean() + colvals.mean()


def kernel(srcs, tgts):
    import concourse.bass_utils as bass_utils

    srcs = np.asarray(srcs, dtype=np.float32)
    tgts = np.asarray(tgts, dtype=np.float32)
    B = srcs.shape[0]
    assert srcs.shape == (B, _D, _N) and tgts.shape == (B, _D, _N)

    nc = _get_program()

    in_maps = []
    host_ctx = []
    for b in range(B):
        W, R, bvec, beta, rs, rt = _host_prep(srcs[b], tgts[b])
        in_maps.append({"w": W, "r": R, "bv": bvec})
        host_ctx.append((beta, rs, rt))

    res = None
    for attempt in range(3):
        try:
            res = bass_utils.run_bass_kernel_spmd(
                nc, in_maps, core_ids=list(range(_NCORES)),
                trace=TRACE, trace_cores=TRACE_CORES if TRACE else None,
            )
            break
        except Exception:
            # transient NRT/device hiccups have been observed; retry
            if attempt == 2:
                raise
            import time
            time.sleep(3.0)
    global LAST_RESULTS
    LAST_RESULTS = res

    total = 0.0
    for b in range(B):
        beta, rs, rt = host_ctx[b]
        total += _host_post(res.results[b], beta, rs, rt,
                            srcs[b], tgts[b])

    return np.float32(total / B)


# revision 16
# speedup vs baseline: 1.0130x; 1.0130x over previous
"""Chamfer-distance (CDLoss) Trainium2 Bass kernel — exp-remap softmin hybrid.

Problem: srcs, tgts [B=8, D=3, N=4096] fp32.
  P[b,i,j] = |s_i|^2 + |t_j|^2 - 2 s_i.t_j
  out = min(P, axis=1).mean() + min(P, axis=2).mean()   (scalar fp32)

Strategy (data-parallel over B across 8 NeuronCores, one batch per core):
  The 4096x4096 distance matrix is produced tile-by-tile on TensorE
  (bf16 hi/lo-split features, K=18, [128,2048]-column units, PSUM f32)
  exactly like the classical baseline.  The classical bottleneck was
  VectorE (row+col min-trees, ~147us busy); this version moves the row
  reduction into the Activation engine's PSUM drain:

  * Act applies E = exp((beta - P)/T) (bias=beta/T, scale=-1/T) while
    draining PSUM -> SBUF bf16.  Cost is identical to the plain cast
    (measured 1967ns/tile; the exp table is warmed at t=0), and
    `accum_out` yields per-partition row sums Sum_j E for free
    (+182ns/tile): row softmin = beta - T*ln(sum), computed on host.
    The row min-tree is gone.
  * The col direction stays a classical pair tree, but in E-space: E
    is a monotone remap of P, so col max of E == col min of P (exact
    to bf16 rounding).  VectorE maxes the 7 exp'd tiles per block into
    a [128,2048] slab that is shipped to DRAM as-is; the host does the
    final max over the 8 slabs x 128 partitions (no device transpose).
  * 12 "direct" units (j==0 every block, plus j==4 in odd blocks;
    m in {0,8,12,16,24,28} x both halves) skip Act: VectorE drains
    their PSUM directly with a 32:1+64:1 tensor_reduce min (rows,
    exact f16) and a mixed f32/f16 TT min into a raw per-half col
    accumulator, rebalancing ~25us of Act work onto VectorE slack.
    The previous block's col-tail group is emitted after the j==0
    drain, so drains never queue behind DVE ops that wait on Act
    (engine queues are in-order).  The LAST block accumulates its col
    max incrementally so only one 1.2us op trails the final exp.

  Numerics: T=1e-3.  exp underflow flushes far pairs (harmless for
  min); rows/cols whose true min exceeds beta+~85T can flush/overflow
  - they self-detect on host (nonfinite / threshold) and are
  recomputed exactly there (~160 rows + ~160 cols per batch, ~60
  Mflops).  Measured end-to-end rel err 2.9e-3 vs the 2e-2 gate.

  Per-core outputs:
    rs_acc [128, 64]    f32  row exp-sums per (h,m) unit (exp'd units)
    rdir   [128, 12]    f16  exact row mins of the direct units
    colE   [128, 16384] bf16 per-(half,block) col-max-of-E slabs
    colR   [128, 4096]  f16  per-half raw col mins over direct units

  HW exec ~151us warm (from the 209us/176us classical baseline); Act
  is the pacing engine (~95% busy), VectorE ~74%.  Beware ~20%
  run-to-run HW variance (p-state).
"""

import numpy as np
import ml_dtypes

_BF16 = ml_dtypes.bfloat16

# Problem geometry (hardcoded per contest contract).
_B = 8
_D = 3
_N = 4096
_P = 128              # partitions / sources per m-tile
_K = 18               # feature rows (see _features)
_NCORES = 8
_CHUNK = 2048         # PSUM chunk columns (4 banks) == column half

_D12 = True           # 12 direct units (j0 all blocks + j4 odd blocks)
_T = 1.0e-3           # softmin temperature
_SCALE = -1.0 / _T
_ROW_THR = 85.0 * _T  # host fallback threshold above beta
_COL_THR = 80.0 * _T

_prog_cache = {}

# test-harness knobs (the grading harness just calls kernel() and never
# touches these; default is the fast no-trace path)
TRACE = False
TRACE_CORES = [0]
LAST_RESULTS = None


def _build_program(n_pts=_N):
    import concourse.mybir as mybir
    import concourse.tile as tile
    from concourse import bacc

    P = _P
    MT = n_pts // P              # 32 m-tiles of 128 sources
    K = _K
    NH = n_pts // _CHUNK         # 2 column halves
    NBLK = MT // 8               # 4 eight-unit blocks per half
    f32 = mybir.dt.float32
    f16 = mybir.dt.float16
    bf16 = mybir.dt.bfloat16
    MIN = mybir.AluOpType.min
    MAX = mybir.AluOpType.max
    AX = mybir.AxisListType.X
    EXP = mybir.ActivationFunctionType.Exp

    nc = bacc.Bacc("TRN2", target_bir_lowering=False, debug=False,
                   num_devices=_NCORES)

    dram_w = nc.dram_tensor("w", [K, n_pts], bf16, kind="ExternalInput")
    dram_r = nc.dram_tensor("r", [K, n_pts], bf16, kind="ExternalInput")
    dram_bv = nc.dram_tensor("bv", [P, 1], f32, kind="ExternalInput")
    dram_rs = nc.dram_tensor("rs_acc", [P, NH * MT], f32,
                             kind="ExternalOutput")
    nrd = 12 if _D12 else 8
    dram_rd = nc.dram_tensor("rdir", [P, nrd], f16,
                             kind="ExternalOutput")
    dram_ce = nc.dram_tensor("colE", [P, NH * NBLK * _CHUNK], bf16,
                             kind="ExternalOutput")
    dram_cr = nc.dram_tensor("colR", [P, n_pts], f16,
                             kind="ExternalOutput")

    with tile.TileContext(nc) as tc:
        with (
            tc.tile_pool(name="const", bufs=1) as cpool,
            tc.tile_pool(name="ring", bufs=2) as ringpool,
            tc.tile_pool(name="stage", bufs=1) as spool,
            tc.tile_pool(name="outs", bufs=1) as rpool,
            tc.tile_pool(name="psum", bufs=2, space="PSUM") as ppool,
        ):
            # Prologue: feature loads replicated into 4 PE row groups so
            # consecutive matmuls rotate groups (LDWEIGHTS overlaps
            # MATMUL).  Priority slices cover unit 0's operands so its
            # matmuls start early; bulk loads follow.  The Act queue is
            # kept DMA-free (Act is the bottleneck engine).
            sbW = cpool.tile([128, n_pts], bf16, tag="sbW", name="sbW")
            sbR = cpool.tile([128, n_pts], bf16, tag="sbR", name="sbR")
            bv = cpool.tile([P, 1], f32, tag="bv", name="bv")
            warm = cpool.tile([P, 1], f32, tag="warm", name="warm")
            nc.sync.dma_start(sbW[0:K, 0:2 * P], dram_w[:, 0:2 * P])
            nc.scalar.dma_start(sbR[0:K, 0:512], dram_r[:, 0:512])
            nc.scalar.dma_start(sbR[0:K, 512:_CHUNK],
                                dram_r[:, 512:_CHUNK])
            nc.sync.dma_start(bv[:], dram_bv[:])
            # warm the Act exp table at t=0 so the first real exp
            # doesn't eat the 1.3us ACT_TABLE_LOAD
            nc.vector.memset(warm[:], 0.0)
            nc.scalar.activation(warm[:], warm[:],
                                 mybir.ActivationFunctionType.Exp,
                                 bias=0.0, scale=1.0)
            nc.sync.dma_start(sbW[32 + 0:32 + K, :], dram_w[:])
            nc.sync.dma_start(sbR[32 + 0:32 + K, :], dram_r[:])
            for g in range(2, 4):
                nc.sync.dma_start(sbW[32 * g:32 * g + K, :], dram_w[:])
                nc.sync.dma_start(sbR[32 * g:32 * g + K, :], dram_r[:])
            nc.sync.dma_start(sbW[0:K, 2 * P:], dram_w[:, 2 * P:])
            nc.sync.dma_start(sbR[0:K, _CHUNK:], dram_r[:, _CHUNK:])

            V = nc.vector
            S = nc.scalar

            RS = rpool.tile([P, NH * MT], f32, tag="RS", name="RS")
            RD = rpool.tile([P, nrd], f16, tag="RD", name="RD")
            CR = rpool.tile([P, n_pts], f16, tag="CR", name="CR")

            pending_tail = [None]

            def emit_tail():
                if pending_tail[0] is not None:
                    pending_tail[0]()
                    pending_tail[0] = None

            # direct units per block: j==0 always; odd blocks also j==4
            # (d=12 total -> Act and Vector both ~balanced).  The
            # per-block col-tail group (C2b + max tree + slab DMA) is
            # deferred until just after the NEXT block's first drain so
            # the drain never queues behind ops that wait on Act.
            dir_ctr = [0]
            for h in range(NH):
                cr_h = CR[:, h * _CHUNK:(h + 1) * _CHUNK]
                for blk in range(NBLK):
                    dirs = (0, 4) if (_D12 and blk % 2 == 1) else (0,)
                    expjs = [j for j in range(8) if j not in dirs]
                    ring0 = ringpool.tile([P, 4, _CHUNK], bf16,
                                          tag="ring0", name="ring0")
                    nring1 = len(expjs) - 4
                    ring1 = ringpool.tile([P, nring1, _CHUNK], bf16,
                                          tag=f"ring1_{nring1}",
                                          name="ring1")
                    C2a = ringpool.tile([P, 2, _CHUNK], bf16,
                                        tag="C2a", name="C2a")
                    CCs = ringpool.tile([P, _CHUNK], bf16, tag="CCs",
                                        name="CCs")
                    first = h == 0 and blk == 0
                    last = h == NH - 1 and blk == NBLK - 1
                    # last block: incremental col max so only one max op
                    # (not the whole tail group) follows the final exp
                    Xl = None
                    if last:
                        Xl = spool.tile([P, _CHUNK], bf16, tag="Xl",
                                        name="Xl")
                    nexp = [0]
                    firstdir = blk == 0
                    for j in range(8):
                        m = blk * 8 + j
                        u = h * MT + m
                        ps = ppool.tile([P, 64, 32], f32, tag="ps")
                        for q in range(4):
                            # row group rotates per 512-col matmul so
                            # LDWEIGHTS overlaps in-flight MATMULs.  In
                            # the very first block each unit pins one
                            # group so unit j depends only on the j-th
                            # prologue DMA pair.
                            g = j if first and j < 4 else q
                            col = _CHUNK * h + 512 * q
                            nc.tensor.matmul(
                                ps[:, 16 * q:16 * (q + 1), :],
                                sbW[32 * g:32 * g + K, m * P:(m + 1) * P],
                                sbR[32 * g:32 * g + K, col:col + 512],
                                start=True, stop=True,
                                tile_position=(32 * g, 0),
                            )
                        if j in dirs:
                            # direct unit: VectorE drains PSUM.  Rows:
                            # 32:1 reduce then 64:1 (exact min, f16).
                            # Cols: mixed f32/f16 TT min into the raw
                            # per-half accumulator (copy to init).
                            diridx = dir_ctr[0]
                            dir_ctr[0] += 1
                            rd1 = spool.tile([P, 64], f16, tag="rd1")
                            V.tensor_reduce(rd1[:], ps[:], axis=AX, op=MIN)
                            if firstdir:
                                V.tensor_copy(cr_h, ps[:, :, :])
                                firstdir = False
                            else:
                                V.tensor_tensor(cr_h, ps[:, :, :], cr_h,
                                                op=MIN)
                            V.tensor_reduce(RD[:, diridx:diridx + 1],
                                            rd1[:], axis=AX, op=MIN)
                            if j == 0:
                                emit_tail()
                        else:
                            k = expjs.index(j)
                            ring, jj = (ring0, k) if k < 4 else (ring1,
                                                                 k - 4)
                            S.activation(ring[:, jj, :], ps[:, :, :], EXP,
                                         bias=bv[:], scale=_SCALE,
                                         accum_out=RS[:, u:u + 1])
                            if last:
                                nexp[0] += 1
                                if nexp[0] == 2:
                                    V.tensor_tensor(Xl[:], ring0[:, 0, :],
                                                    ring0[:, 1, :],
                                                    op=MAX)
                                elif nexp[0] > 2:
                                    src_t = (ring0[:, k, :] if k < 4
                                             else ring1[:, k - 4, :])
                                    V.tensor_tensor(Xl[:], Xl[:], src_t,
                                                    op=MAX)
                        if not last and j == expjs[3]:
                            V.tensor_tensor(C2a[:], ring0[:, 0:4:2, :],
                                            ring0[:, 1:4:2, :], op=MAX)
                    if last:
                        slab = (h * NBLK + blk) * _CHUNK
                        nc.sync.dma_start(dram_ce[:, slab:slab + _CHUNK],
                                          Xl[:])
                        continue
                    # tail group: C2b + max tree + slab DMA, deferred
                    def make_tail(h=h, blk=blk, nring1=nring1,
                                  ring1=ring1, C2a=C2a, CCs=CCs):
                        def tail():
                            C2b = spool.tile([P, _CHUNK], bf16,
                                             tag="C2b")
                            C4a = spool.tile([P, _CHUNK], bf16,
                                             tag="C4a")
                            V.tensor_tensor(C2b[:], ring1[:, 0, :],
                                            ring1[:, 1, :], op=MAX)
                            V.tensor_tensor(C4a[:], C2a[:, 0, :],
                                            C2a[:, 1, :], op=MAX)
                            if nring1 == 3:
                                C4b = spool.tile([P, _CHUNK], bf16,
                                                 tag="C4b")
                                V.tensor_tensor(C4b[:], C2b[:],
                                                ring1[:, 2, :], op=MAX)
                                V.tensor_tensor(CCs[:], C4a[:], C4b[:],
                                                op=MAX)
                            else:
                                V.tensor_tensor(CCs[:], C4a[:], C2b[:],
                                                op=MAX)
                            slab = (h * NBLK + blk) * _CHUNK
                            nc.sync.dma_start(
                                dram_ce[:, slab:slab + _CHUNK], CCs[:])
                        return tail
                    pending_tail[0] = make_tail()
                nc.sync.dma_start(dram_cr[:, h * _CHUNK:(h + 1) * _CHUNK],
                                  cr_h)
            emit_tail()
            nc.sync.dma_start(dram_rs[:], RS[:])
            nc.sync.dma_start(dram_rd[:], RD[:])

    nc.compile()
    return nc


def _get_program(n_pts=_N):
    if n_pts not in _prog_cache:
        _prog_cache[n_pts] = _build_program(n_pts)
    return _prog_cache[n_pts]


def _split_bf16(x32):
    """x32 fp32 -> (hi, lo) bf16 with hi+lo ~= x to ~2^-18 rel."""
    hi = x32.astype(_BF16)
    lo = (x32 - hi.astype(np.float32)).astype(_BF16)
    return hi, lo


def _split3(x64):
    """fp64 vector -> 3 bf16 terms summing to x to ~2^-27 rel."""
    t0 = x64.astype(_BF16)
    r = x64 - t0.astype(np.float64)
    t1 = r.astype(_BF16)
    r2 = r - t1.astype(np.float64)
    t2 = r2.astype(_BF16)
    return t0, t1, t2


def _features(q, c, n_pts):
    """Feature tensors for the distance matmul.

    q: query points  [3, N] fp32; c: candidate points [3, N] fp32.
    Returns (W [18, N] bf16, R [18, N] bf16) with
      (W.T @ R)[i, j] ~= |q~_i - c~_j|^2
    with ~ the bf16-split (hi+lo) values, exact to ~2e-6.
    """
    q_hi, q_lo = _split_bf16(q)
    c_hi, c_lo = _split_bf16(c)
    q_t = q_hi.astype(np.float32) + q_lo.astype(np.float32)
    c_t = c_hi.astype(np.float32) + c_lo.astype(np.float32)

    U = (c_t.astype(np.float64) ** 2).sum(axis=0)   # candidate norms
    u0, u1, u2 = _split3(U)
    V = (q_t.astype(np.float64) ** 2).sum(axis=0)   # query norms
    v0, v1, v2 = _split3(V)

    m2q_hi = (-2.0 * q_hi.astype(np.float32)).astype(_BF16)
    m2q_lo = (-2.0 * q_lo.astype(np.float32)).astype(_BF16)
    ones = np.ones(n_pts, dtype=_BF16)

    Wg = np.concatenate([
        m2q_hi, m2q_hi, m2q_lo, m2q_lo,
        np.stack([ones, ones, ones]),
        np.stack([v0, v1, v2]),
    ], axis=0).astype(_BF16)              # [18, N]
    Rg = np.concatenate([
        c_hi, c_lo, c_hi, c_lo,
        np.stack([u0, u1, u2]),
        np.stack([ones, ones, ones]),
    ], axis=0).astype(_BF16)              # [18, N]

    return Wg, Rg


def _host_prep(srcs_b, tgts_b):
    """Features + global softmin bias beta for one batch."""
    W, R = _features(srcs_b, tgts_b, _N)
    s = srcs_b.astype(np.float64)          # [3, N]
    t = tgts_b.astype(np.float64)
    rs = (s * s).sum(0)                    # |s_i|^2
    rt = (t * t).sum(0)
    # subsample estimate of per-row mins (stride 4 -> 1024 targets)
    tsub = t[:, ::4]
    cross = s.T @ tsub                     # [N, N/4]
    psub = rs[:, None] + rt[None, ::4] - 2.0 * cross
    bi = psub.min(axis=1)
    beta = float(bi.min())
    bvec = np.full((_P, 1), beta / _T, dtype=np.float32)
    return W, R, bvec, beta, rs, rt


def _host_post(res_b, beta, rs, rt, srcs_b, tgts_b):
    """Recover row/col minima for one batch; exact host fallback for
    flushed/overflowed entries."""
    s = srcs_b.astype(np.float64)
    t = tgts_b.astype(np.float64)
    MT = _N // _P

    rs_acc = res_b["rs_acc"].astype(np.float64)   # [128, 64]
    rdir = res_b["rdir"].astype(np.float64)
    colE = res_b["colE"].astype(np.float64)       # [128, 8*2048]
    colR = res_b["colR"].astype(np.float64)       # [128, 4096]

    # --- rows ---
    M_DIR = [0, 8, 12, 16, 24, 28] if _D12 else [0, 8, 16, 24]
    rowvals = np.empty(_N)
    bad_rows = []
    for m in range(MT):
        sl = slice(m * _P, (m + 1) * _P)
        if m in M_DIR:
            d = M_DIR.index(m)
            nd = len(M_DIR)
            rowvals[sl] = np.minimum(rdir[:, d], rdir[:, nd + d])
        else:
            Rsum = rs_acc[:, m] + rs_acc[:, MT + m]
            with np.errstate(divide="ignore", invalid="ignore"):
                v = beta - _T * np.log(Rsum)
            bad = ~np.isfinite(v) | (v > beta + _ROW_THR)
            v = np.where(bad, np.inf, v)
            rowvals[sl] = v
            if bad.any():
                bad_rows.extend(m * _P + np.nonzero(bad)[0])
    if bad_rows:
        idx = np.asarray(bad_rows)
        d2 = (rs[idx, None] + rt[None, :]
              - 2.0 * (s[:, idx].T @ t))
        rowvals[idx] = d2.min(axis=1)

    # --- cols ---
    # colE slabs: (h*4+blk)*2048 + c -> target column 2048h + c
    slabs = colE.reshape(128, 2, 4, _CHUNK)       # [p, h, blk, c]
    maxE = slabs.max(axis=(0, 2)).reshape(-1)     # [4096]
    with np.errstate(divide="ignore", invalid="ignore"):
        vE = beta - _T * np.log(maxE)
    vR = colR.min(axis=0)
    colvals = np.minimum(np.where(np.isfinite(vE), vE, np.inf), vR)
    bad_c = ~np.isfinite(colvals) | (colvals > beta + _COL_THR)
    if bad_c.any():
        jdx = np.nonzero(bad_c)[0]
        d2 = (rs[:, None] + rt[jdx][None, :]
              - 2.0 * (s.T @ t[:, jdx]))
        colvals[jdx] = d2.min(axis=0)

    return rowvals.mean() + colvals.mean()


def kernel(srcs, tgts):
    import concourse.bass_utils as bass_utils

    srcs = np.asarray(srcs, dtype=np.float32)
    tgts = np.asarray(tgts, dtype=np.float32)
    B = srcs.shape[0]
    assert srcs.shape == (B, _D, _N) and tgts.shape == (B, _D, _N)

    nc = _get_program()

    in_maps = []
    host_ctx = []
    for b in range(B):
        W, R, bvec, beta, rs, rt = _host_prep(srcs[b], tgts[b])
        in_maps.append({"w": W, "r": R, "bv": bvec})
        host_ctx.append((beta, rs, rt))

    res = None
    for attempt in range(3):
        try:
            res = bass_utils.run_bass_kernel_spmd(
                nc, in_maps, core_ids=list(range(_NCORES)),
                trace=TRACE, trace_cores=TRACE_CORES if TRACE else None,
            )
            break
        except Exception:
            # transient NRT/device hiccups have been observed; retry
            if attempt == 2:
                raise
            import time
            time.sleep(3.0)
    global LAST_RESULTS
    LAST_RESULTS = res

    total = 0.0
    for b in range(B):
        beta, rs, rt = host_ctx[b]
        total += _host_post(res.results[b], beta, rs, rt,
                            srcs[b], tgts[b])

    return np.float32(total / B)


# revision 17
# speedup vs baseline: 1.0145x; 1.0015x over previous
"""Chamfer-distance (CDLoss) Trainium2 Bass kernel — exp-remap softmin hybrid.

Problem: srcs, tgts [B=8, D=3, N=4096] fp32.
  P[b,i,j] = |s_i|^2 + |t_j|^2 - 2 s_i.t_j
  out = min(P, axis=1).mean() + min(P, axis=2).mean()   (scalar fp32)

Strategy (data-parallel over B across 8 NeuronCores, one batch per core):
  The 4096x4096 distance matrix is produced tile-by-tile on TensorE
  (bf16 hi/lo-split features, K=18, [128,2048]-column units, PSUM f32)
  exactly like the classical baseline.  The classical bottleneck was
  VectorE (row+col min-trees, ~147us busy); this version moves the row
  reduction into the Activation engine's PSUM drain:

  * Act applies E = exp((beta - P)/T) (bias=beta/T, scale=-1/T) while
    draining PSUM -> SBUF bf16.  Cost is identical to the plain cast
    (measured 1967ns/tile; the exp table is warmed at t=0), and
    `accum_out` yields per-partition row sums Sum_j E for free
    (+182ns/tile): row softmin = beta - T*ln(sum), computed on host.
    The row min-tree is gone.
  * The col direction stays a classical pair tree, but in E-space: E
    is a monotone remap of P, so col max of E == col min of P (exact
    to bf16 rounding).  VectorE maxes the 7 exp'd tiles per block into
    a [128,2048] slab that is shipped to DRAM as-is; the host does the
    final max over the 8 slabs x 128 partitions (no device transpose).
  * 12 "direct" units (j==0 every block, plus j==4 in odd blocks;
    m in {0,8,12,16,24,28} x both halves) skip Act: VectorE drains
    their PSUM directly with a 32:1+64:1 tensor_reduce min (rows,
    exact f16) and a mixed f32/f16 TT min into a raw per-half col
    accumulator, rebalancing ~25us of Act work onto VectorE slack.
    The previous block's col-tail group is emitted after the j==0
    drain, so drains never queue behind DVE ops that wait on Act
    (engine queues are in-order).  The LAST block accumulates its col
    max incrementally so only one 1.2us op trails the final exp.

  Numerics: T=1e-3.  exp underflow flushes far pairs (harmless for
  min); rows/cols whose true min exceeds beta+~85T can flush/overflow
  - they self-detect on host (nonfinite / threshold) and are
  recomputed exactly there (~160 rows + ~160 cols per batch, ~60
  Mflops).  Measured end-to-end rel err 2.9e-3 vs the 2e-2 gate.

  Per-core outputs:
    rs_acc [128, 64]    f32  row exp-sums per (h,m) unit (exp'd units)
    rdir   [128, 12]    f16  exact row mins of the direct units
    colE   [128, 16384] bf16 per-(half,block) col-max-of-E slabs
    colR   [128, 4096]  f16  per-half raw col mins over direct units

  HW exec ~151us warm (from the 209us/176us classical baseline); Act
  is the pacing engine (~95% busy), VectorE ~74%.  Beware ~20%
  run-to-run HW variance (p-state).
"""

import numpy as np
import ml_dtypes

_BF16 = ml_dtypes.bfloat16

# Problem geometry (hardcoded per contest contract).
_B = 8
_D = 3
_N = 4096
_P = 128              # partitions / sources per m-tile
_K = 18               # feature rows (see _features)
_NCORES = 8
_CHUNK = 2048         # PSUM chunk columns (4 banks) == column half

_D12 = True           # 12 direct units (j0 all blocks + j4 odd blocks)
_T = 1.0e-3           # softmin temperature
_SCALE = -1.0 / _T
_ROW_THR = 85.0 * _T  # host fallback threshold above beta
_COL_THR = 80.0 * _T

_prog_cache = {}

# test-harness knobs (the grading harness just calls kernel() and never
# touches these; default is the fast no-trace path)
TRACE = False
TRACE_CORES = [0]
LAST_RESULTS = None


def _build_program(n_pts=_N):
    import concourse.mybir as mybir
    import concourse.tile as tile
    from concourse import bacc

    P = _P
    MT = n_pts // P              # 32 m-tiles of 128 sources
    K = _K
    NH = n_pts // _CHUNK         # 2 column halves
    NBLK = MT // 8               # 4 eight-unit blocks per half
    f32 = mybir.dt.float32
    f16 = mybir.dt.float16
    bf16 = mybir.dt.bfloat16
    MIN = mybir.AluOpType.min
    MAX = mybir.AluOpType.max
    AX = mybir.AxisListType.X
    EXP = mybir.ActivationFunctionType.Exp

    nc = bacc.Bacc("TRN2", target_bir_lowering=False, debug=False,
                   num_devices=_NCORES)

    dram_w = nc.dram_tensor("w", [K, n_pts], bf16, kind="ExternalInput")
    dram_r = nc.dram_tensor("r", [K, n_pts], bf16, kind="ExternalInput")
    dram_bv = nc.dram_tensor("bv", [P, 1], f32, kind="ExternalInput")
    dram_rs = nc.dram_tensor("rs_acc", [P, NH * MT], f32,
                             kind="ExternalOutput")
    nrd = 12 if _D12 else 8
    dram_rd = nc.dram_tensor("rdir", [P, nrd], f16,
                             kind="ExternalOutput")
    dram_ce = nc.dram_tensor("colE", [P, NH * NBLK * _CHUNK], bf16,
                             kind="ExternalOutput")
    dram_cr = nc.dram_tensor("colR", [P, n_pts], f16,
                             kind="ExternalOutput")

    with tile.TileContext(nc) as tc:
        with (
            tc.tile_pool(name="const", bufs=1) as cpool,
            tc.tile_pool(name="ring", bufs=2) as ringpool,
            tc.tile_pool(name="stage", bufs=1) as spool,
            tc.tile_pool(name="outs", bufs=1) as rpool,
            tc.tile_pool(name="psum", bufs=2, space="PSUM") as ppool,
        ):
            # Prologue: feature loads replicated into 4 PE row groups so
            # consecutive matmuls rotate groups (LDWEIGHTS overlaps
            # MATMUL).  Priority slices cover unit 0's operands so its
            # matmuls start early; bulk loads follow.  The Act queue is
            # kept DMA-free (Act is the bottleneck engine).
            sbW = cpool.tile([128, n_pts], bf16, tag="sbW", name="sbW")
            sbR = cpool.tile([128, n_pts], bf16, tag="sbR", name="sbR")
            bv = cpool.tile([P, 1], f32, tag="bv", name="bv")
            warm = cpool.tile([P, 1], f32, tag="warm", name="warm")
            nc.sync.dma_start(sbW[0:K, 0:2 * P], dram_w[:, 0:2 * P])
            nc.scalar.dma_start(sbR[0:K, 0:512], dram_r[:, 0:512])
            nc.scalar.dma_start(sbR[0:K, 512:_CHUNK],
                                dram_r[:, 512:_CHUNK])
            nc.sync.dma_start(bv[:], dram_bv[:])
            # warm the Act exp table at t=0 so the first real exp
            # doesn't eat the 1.3us ACT_TABLE_LOAD
            nc.vector.memset(warm[:], 0.0)
            nc.scalar.activation(warm[:], warm[:],
                                 mybir.ActivationFunctionType.Exp,
                                 bias=0.0, scale=1.0)
            nc.sync.dma_start(sbW[32 + 0:32 + K, :], dram_w[:])
            nc.sync.dma_start(sbR[32 + 0:32 + K, :], dram_r[:])
            for g in range(2, 4):
                nc.sync.dma_start(sbW[32 * g:32 * g + K, :], dram_w[:])
                nc.sync.dma_start(sbR[32 * g:32 * g + K, :], dram_r[:])
            nc.sync.dma_start(sbW[0:K, 2 * P:], dram_w[:, 2 * P:])
            nc.sync.dma_start(sbR[0:K, _CHUNK:], dram_r[:, _CHUNK:])

            V = nc.vector
            S = nc.scalar

            RS = rpool.tile([P, NH * MT], f32, tag="RS", name="RS")
            RD = rpool.tile([P, nrd], f16, tag="RD", name="RD")
            CR = rpool.tile([P, n_pts], f16, tag="CR", name="CR")

            pending_tail = [None]

            def emit_tail():
                if pending_tail[0] is not None:
                    pending_tail[0]()
                    pending_tail[0] = None

            # direct units per block: j==0 always; odd blocks also j==4
            # (d=12 total -> Act and Vector both ~balanced).  The
            # per-block col-tail group (C2b + max tree + slab DMA) is
            # deferred until just after the NEXT block's first drain so
            # the drain never queues behind ops that wait on Act.
            dir_ctr = [0]
            for h in range(NH):
                cr_h = CR[:, h * _CHUNK:(h + 1) * _CHUNK]
                for blk in range(NBLK):
                    dirs = (0, 4) if (_D12 and blk % 2 == 1) else (0,)
                    expjs = [j for j in range(8) if j not in dirs]
                    ring0 = ringpool.tile([P, 4, _CHUNK], bf16,
                                          tag="ring0", name="ring0")
                    nring1 = len(expjs) - 4
                    ring1 = ringpool.tile([P, nring1, _CHUNK], bf16,
                                          tag=f"ring1_{nring1}",
                                          name="ring1")
                    C2a = ringpool.tile([P, 2, _CHUNK], bf16,
                                        tag="C2a", name="C2a")
                    CCs = ringpool.tile([P, _CHUNK], bf16, tag="CCs",
                                        name="CCs")
                    first = h == 0 and blk == 0
                    last = h == NH - 1 and blk == NBLK - 1
                    # last block: incremental col max so only one max op
                    # (not the whole tail group) follows the final exp
                    Xl = None
                    if last:
                        Xl = spool.tile([P, _CHUNK], bf16, tag="Xl",
                                        name="Xl")
                    nexp = [0]
                    firstdir = blk == 0
                    for j in range(8):
                        m = blk * 8 + j
                        u = h * MT + m
                        ps = ppool.tile([P, 64, 32], f32, tag="ps")
                        for q in range(4):
                            # row group rotates per 512-col matmul so
                            # LDWEIGHTS overlaps in-flight MATMULs.  In
                            # the very first block each unit pins one
                            # group so unit j depends only on the j-th
                            # prologue DMA pair.
                            g = j if first and j < 4 else q
                            col = _CHUNK * h + 512 * q
                            nc.tensor.matmul(
                                ps[:, 16 * q:16 * (q + 1), :],
                                sbW[32 * g:32 * g + K, m * P:(m + 1) * P],
                                sbR[32 * g:32 * g + K, col:col + 512],
                                start=True, stop=True,
                                tile_position=(32 * g, 0),
                            )
                        if j in dirs:
                            # direct unit: VectorE drains PSUM.  Rows:
                            # 32:1 reduce then 64:1 (exact min, f16).
                            # Cols: mixed f32/f16 TT min into the raw
                            # per-half accumulator (copy to init).
                            diridx = dir_ctr[0]
                            dir_ctr[0] += 1
                            rd1 = spool.tile([P, 64], f16, tag="rd1")
                            V.tensor_reduce(rd1[:], ps[:], axis=AX, op=MIN)
                            if firstdir:
                                V.tensor_copy(cr_h, ps[:, :, :])
                                firstdir = False
                            else:
                                V.tensor_tensor(cr_h, ps[:, :, :], cr_h,
                                                op=MIN)
                            V.tensor_reduce(RD[:, diridx:diridx + 1],
                                            rd1[:], axis=AX, op=MIN)
                            if j == 0:
                                emit_tail()
                        else:
                            k = expjs.index(j)
                            ring, jj = (ring0, k) if k < 4 else (ring1,
                                                                 k - 4)
                            S.activation(ring[:, jj, :], ps[:, :, :], EXP,
                                         bias=bv[:], scale=_SCALE,
                                         accum_out=RS[:, u:u + 1])
                            if last:
                                nexp[0] += 1
                                if nexp[0] == 2:
                                    V.tensor_tensor(Xl[:], ring0[:, 0, :],
                                                    ring0[:, 1, :],
                                                    op=MAX)
                                elif nexp[0] > 2:
                                    src_t = (ring0[:, k, :] if k < 4
                                             else ring1[:, k - 4, :])
                                    V.tensor_tensor(Xl[:], Xl[:], src_t,
                                                    op=MAX)
                        if not last and j == expjs[3]:
                            V.tensor_tensor(C2a[:], ring0[:, 0:4:2, :],
                                            ring0[:, 1:4:2, :], op=MAX)
                    if last:
                        slab = (h * NBLK + blk) * _CHUNK
                        nc.sync.dma_start(dram_ce[:, slab:slab + _CHUNK],
                                          Xl[:])
                        continue
                    # tail group: C2b + max tree + slab DMA, deferred
                    def make_tail(h=h, blk=blk, nring1=nring1,
                                  ring1=ring1, C2a=C2a, CCs=CCs):
                        def tail():
                            C2b = spool.tile([P, _CHUNK], bf16,
                                             tag="C2b")
                            C4a = spool.tile([P, _CHUNK], bf16,
                                             tag="C4a")
                            V.tensor_tensor(C2b[:], ring1[:, 0, :],
                                            ring1[:, 1, :], op=MAX)
                            V.tensor_tensor(C4a[:], C2a[:, 0, :],
                                            C2a[:, 1, :], op=MAX)
                            if nring1 == 3:
                                C4b = spool.tile([P, _CHUNK], bf16,
                                                 tag="C4b")
                                V.tensor_tensor(C4b[:], C2b[:],
                                                ring1[:, 2, :], op=MAX)
                                V.tensor_tensor(CCs[:], C4a[:], C4b[:],
                                                op=MAX)
                            else:
                                V.tensor_tensor(CCs[:], C4a[:], C2b[:],
                                                op=MAX)
                            slab = (h * NBLK + blk) * _CHUNK
                            nc.sync.dma_start(
                                dram_ce[:, slab:slab + _CHUNK], CCs[:])
                        return tail
                    pending_tail[0] = make_tail()
                nc.sync.dma_start(dram_cr[:, h * _CHUNK:(h + 1) * _CHUNK],
                                  cr_h)
                # ship this half's row accums now: only the second
                # half's 16KB RS slice then trails the final exp
                nc.sync.dma_start(dram_rs[:, h * MT:(h + 1) * MT],
                                  RS[:, h * MT:(h + 1) * MT])
            emit_tail()
            nc.sync.dma_start(dram_rd[:], RD[:])

    nc.compile()
    return nc


def _get_program(n_pts=_N):
    if n_pts not in _prog_cache:
        _prog_cache[n_pts] = _build_program(n_pts)
    return _prog_cache[n_pts]


def _split_bf16(x32):
    """x32 fp32 -> (hi, lo) bf16 with hi+lo ~= x to ~2^-18 rel."""
    hi = x32.astype(_BF16)
    lo = (x32 - hi.astype(np.float32)).astype(_BF16)
    return hi, lo


def _split3(x64):
    """fp64 vector -> 3 bf16 terms summing to x to ~2^-27 rel."""
    t0 = x64.astype(_BF16)
    r = x64 - t0.astype(np.float64)
    t1 = r.astype(_BF16)
    r2 = r - t1.astype(np.float64)
    t2 = r2.astype(_BF16)
    return t0, t1, t2


def _features(q, c, n_pts):
    """Feature tensors for the distance matmul.

    q: query points  [3, N] fp32; c: candidate points [3, N] fp32.
    Returns (W [18, N] bf16, R [18, N] bf16) with
      (W.T @ R)[i, j] ~= |q~_i - c~_j|^2
    with ~ the bf16-split (hi+lo) values, exact to ~2e-6.
    """
    q_hi, q_lo = _split_bf16(q)
    c_hi, c_lo = _split_bf16(c)
    q_t = q_hi.astype(np.float32) + q_lo.astype(np.float32)
    c_t = c_hi.astype(np.float32) + c_lo.astype(np.float32)

    U = (c_t.astype(np.float64) ** 2).sum(axis=0)   # candidate norms
    u0, u1, u2 = _split3(U)
    V = (q_t.astype(np.float64) ** 2).sum(axis=0)   # query norms
    v0, v1, v2 = _split3(V)

    m2q_hi = (-2.0 * q_hi.astype(np.float32)).astype(_BF16)
    m2q_lo = (-2.0 * q_lo.astype(np.float32)).astype(_BF16)
    ones = np.ones(n_pts, dtype=_BF16)

    Wg = np.concatenate([
        m2q_hi, m2q_hi, m2q_lo, m2q_lo,
        np.stack([ones, ones, ones]),
        np.stack([v0, v1, v2]),
    ], axis=0).astype(_BF16)              # [18, N]
    Rg = np.concatenate([
        c_hi, c_lo, c_hi, c_lo,
        np.stack([u0, u1, u2]),
        np.stack([ones, ones, ones]),
    ], axis=0).astype(_BF16)              # [18, N]

    return Wg, Rg


def _host_prep(srcs_b, tgts_b):
    """Features + global softmin bias beta for one batch."""
    W, R = _features(srcs_b, tgts_b, _N)
    s = srcs_b.astype(np.float64)          # [3, N]
    t = tgts_b.astype(np.float64)
    rs = (s * s).sum(0)                    # |s_i|^2
    rt = (t * t).sum(0)
    # subsample estimate of per-row mins (stride 4 -> 1024 targets)
    tsub = t[:, ::4]
    cross = s.T @ tsub                     # [N, N/4]
    psub = rs[:, None] + rt[None, ::4] - 2.0 * cross
    bi = psub.min(axis=1)
    beta = float(bi.min())
    bvec = np.full((_P, 1), beta / _T, dtype=np.float32)
    return W, R, bvec, beta, rs, rt


def _host_post(res_b, beta, rs, rt, srcs_b, tgts_b):
    """Recover row/col minima for one batch; exact host fallback for
    flushed/overflowed entries."""
    s = srcs_b.astype(np.float64)
    t = tgts_b.astype(np.float64)
    MT = _N // _P

    rs_acc = res_b["rs_acc"].astype(np.float64)   # [128, 64]
    rdir = res_b["rdir"].astype(np.float64)
    colE = res_b["colE"].astype(np.float64)       # [128, 8*2048]
    colR = res_b["colR"].astype(np.float64)       # [128, 4096]

    # --- rows ---
    M_DIR = [0, 8, 12, 16, 24, 28] if _D12 else [0, 8, 16, 24]
    rowvals = np.empty(_N)
    bad_rows = []
    for m in range(MT):
        sl = slice(m * _P, (m + 1) * _P)
        if m in M_DIR:
            d = M_DIR.index(m)
            nd = len(M_DIR)
            rowvals[sl] = np.minimum(rdir[:, d], rdir[:, nd + d])
        else:
            Rsum = rs_acc[:, m] + rs_acc[:, MT + m]
            with np.errstate(divide="ignore", invalid="ignore"):
                v = beta - _T * np.log(Rsum)
            bad = ~np.isfinite(v) | (v > beta + _ROW_THR)
            v = np.where(bad, np.inf, v)
            rowvals[sl] = v
            if bad.any():
                bad_rows.extend(m * _P + np.nonzero(bad)[0])
    if bad_rows:
        idx = np.asarray(bad_rows)
        d2 = (rs[idx, None] + rt[None, :]
              - 2.0 * (s[:, idx].T @ t))
        rowvals[idx] = d2.min(axis=1)

    # --- cols ---
    # colE slabs: (h*4+blk)*2048 + c -> target column 2048h + c
    slabs = colE.reshape(128, 2, 4, _CHUNK)       # [p, h, blk, c]
    maxE = slabs.max(axis=(0, 2)).reshape(-1)     # [4096]
    with np.errstate(divide="ignore", invalid="ignore"):
        vE = beta - _T * np.log(maxE)
    vR = colR.min(axis=0)
    colvals = np.minimum(np.where(np.isfinite(vE), vE, np.inf), vR)
    bad_c = ~np.isfinite(colvals) | (colvals > beta + _COL_THR)
    if bad_c.any():
        jdx = np.nonzero(bad_c)[0]
        d2 = (rs[:, None] + rt[jdx][None, :]
              - 2.0 * (s.T @ t[:, jdx]))
        colvals[jdx] = d2.min(axis=0)

    return rowvals.mean() + colvals.mean()


def kernel(srcs, tgts):
    import concourse.bass_utils as bass_utils

    srcs = np.asarray(srcs, dtype=np.float32)
    tgts = np.asarray(tgts, dtype=np.float32)
    B = srcs.shape[0]
    assert srcs.shape == (B, _D, _N) and tgts.shape == (B, _D, _N)

    nc = _get_program()

    in_maps = []
    host_ctx = []
    for b in range(B):
        W, R, bvec, beta, rs, rt = _host_prep(srcs[b], tgts[b])
        in_maps.append({"w": W, "r": R, "bv": bvec})
        host_ctx.append((beta, rs, rt))

    res = None
    for attempt in range(3):
        try:
            res = bass_utils.run_bass_kernel_spmd(
                nc, in_maps, core_ids=list(range(_NCORES)),
                trace=TRACE, trace_cores=TRACE_CORES if TRACE else None,
            )
            break
        except Exception:
            # transient NRT/device hiccups have been observed; retry
            if attempt == 2:
                raise
            import time
            time.sleep(3.0)
    global LAST_RESULTS
    LAST_RESULTS = res

    total = 0.0
    for b in range(B):
        beta, rs, rt = host_ctx[b]
        total += _host_post(res.results[b], beta, rs, rt,
                            srcs[b], tgts[b])

    return np.float32(total / B)
